# revision 1
# baseline (speedup 1.0000x reference)
"""Trainium2 Bass kernel for nn_CurvatureLoss: loss = sum(|lap(pred)-lap(target)| * mask) / (sum(mask)+1e-8).

Sharding: pure data parallel — batch 16 split 2 images per core across 8 cores.
Per-core kernel computes partial sums; host combines and divides.

Math: laplacian is linear, so |lap(pred)-lap(target)| = |lap(d)| with d = pred-target.
The 5-point laplacian of a [128-row, W] tile is computed entirely on the TensorEngine
by accumulating into PSUM:
   psum  = T4^T @ d[:, c]        (vertical tridiag: up + down - 4*center, within block)
         + I^T  @ d[:, c-1]      (left neighbor,  via free-dim offset)
         + I^T  @ d[:, c+1]      (right neighbor, via free-dim offset)
         + Etop^T @ d_prev       (row 127 of block above -> row 0)
         + Ebot^T @ d_next       (row 0 of block below  -> row 127)
d = pred - target is computed in bf16 BY THE LOAD ITSELF (SWDGE CCE
accumulate with fp32->bf16 cast; host passes -target since CCE only adds),
so no compute engine touches pred/target. Per 128-row block: TensorE builds
the laplacian in PSUM, VectorE multiplies by mask and abs-sum-reduces into
per-partition partials, ScalarE abs-accumulates the mask sum. The host sums
the 8 cores' [128, 32] partials in float64 and divides.

End-to-end device time modeled by TimelineSim: ~74.9 us/core, against a
~68.4 us SDMA-bandwidth floor (24 MiB of input per core at the cost
model's 368 GB/s). Key overlap levers: pred DMAs issued two ahead of the
queue-blocking targ accumulates, mask DMAs held behind the d chunks so the
critical d path gets full SDMA bandwidth first, and the kernel-tail barrier
butterflies dropped (nothing left to guard once the sem clear is gone).
"""

import numpy as np
from contextlib import ExitStack

import concourse.bass as bass
import concourse.tile as tile
import concourse.mybir as mybir
from concourse.bass_utils import run_bass_kernel_spmd

F32 = mybir.dt.float32
BF16 = mybir.dt.bfloat16

# Problem constants (hardcoded; kernel.py must be self-contained)
N_CORES = 8
B_TOTAL = 16
B = B_TOTAL // N_CORES  # images per core
H = 1024
W = 1024
P = 128


def make_consts(dtype=np.float32):
    """Stationary matrices, packed [128, 4, 128] (lhsT layout: out = lhsT.T @ rhs)."""
    T4 = np.zeros((P, P), np.float32)
    for k in range(P):
        T4[k, k] = -4.0
        if k > 0:
            T4[k, k - 1] = 1.0
        if k < P - 1:
            T4[k, k + 1] = 1.0
    I = np.eye(P, dtype=np.float32)
    Etop = np.zeros((P, P), np.float32)
    Etop[P - 1, 0] = 1.0  # out[0,:] = rhs[127,:]
    Ebot = np.zeros((P, P), np.float32)
    Ebot[0, P - 1] = 1.0  # out[127,:] = rhs[0,:]
    return np.stack([T4, I, Etop, Ebot], axis=1).astype(dtype)  # [128, 4, 128]


def build_nc(b=B, h=H, w=W):
    """Build the per-core Bass program. Returns nc. Output 'partials' is
    [128, 2*nt*b] fp32: columns [0, nt*b) are masked-abs-lap partial sums,
    columns [nt*b, 2*nt*b) are mask partial sums (both per-partition)."""
    assert h % P == 0 and w % 512 == 0
    nt = h // P
    nidx = nt * b
    # d/mask are loaded in chunks of (image, row-group): DMA APs must be <=3D,
    # so each chunk DMA covers one image's contiguous group of row-blocks.
    ngrp = 2 if nt % 2 == 0 else 1   # row-groups per image
    ct = nt // ngrp                  # row-blocks per group
    nch = b * ngrp                   # total chunks (and mask DMAs); 2*nch SWDGE DMAs
    assert 2 * nch <= 8 and nch + 3 <= 8, "DGE ring budget (8 lanes) exceeded"

    nc = bass.Bass("TRN2", debug=False)

    # The kernel-tail EVENT_SEMAPHORE_RANGE_CLEAR that TileContext emits via
    # clear_and_free_semaphores fails this walrus build's codegen ("ISA wrong
    # length"). Skip just that instruction on this instance; NRT re-zeroes
    # semaphores per execution, so the end-of-kernel clear is not needed for
    # repeated runs (verified empirically in test.py).
    import types
    from concourse.bass import compact_to_ranges

    def _clear_and_free_semaphores(self, sems):
        if not sems:
            return
        sem_nums = [s.num if hasattr(s, "num") else s for s in sems]
        for sem_range in compact_to_ranges(sem_nums):
            assert self._state.free_isdisjoint(sem_range)
            self.gpsimd.dma_reset(sem_range)
        self._state.prepend_free_semaphores(sem_nums)
        for poison_set in self._tile_sem_poison_stack:
            poison_set.update(sem_nums)

    nc.clear_and_free_semaphores = types.MethodType(_clear_and_free_semaphores, nc)

    pred_d = nc.dram_tensor("pred", [b, h, w], F32, kind="ExternalInput")
    targ_d = nc.dram_tensor("target", [b, h, w], F32, kind="ExternalInput")
    mask_d = nc.dram_tensor("mask", [b, h, w], F32, kind="ExternalInput")
    consts_d = nc.dram_tensor("consts", [P, 4, P], BF16, kind="ExternalInput")
    out_d = nc.dram_tensor("partials", [P, 2 * nidx], F32, kind="ExternalOutput")

    pred_ap = pred_d.ap()
    targ_ap = targ_d.ap()
    mask_ap = mask_d.ap()

    # Sync-wait budget: every instruction on this toolchain can carry at most
    # ONE semaphore wait (walrus "Too many sync wait commands" otherwise), and
    # every DMA beyond 8 in flight per DGE ring carries a structural
    # lane-predecessor wait. Design consequences:
    #  - d = pred - target is computed BY THE DMA (SWDGE CCE accumulate with
    #    fp32->bf16 cast), into fresh chunk tiles -> load DMAs carry at most
    #    one WAW wait and never a WAR wait (no slot reuse anywhere).
    #  - <= 8 DMAs per DGE ring total (SWDGE: 2*nch d-chunk DMAs; HWDGE:
    #    consts + nch mask DMAs + 2 output DMAs).
    #  - tiny 1-element "clock carrier" ops absorb DMA-completion ticks onto
    #    consuming engines so real compute ops only ever need one wait.
    # Sign note: the accumulate computes targ - pred = -d; |lap(-d)| = |lap(d)|.
    with tile.TileContext(nc) as tc, ExitStack() as ctx:
        # The kernel-tail drain normally carries one wait per live proc (~18);
        # this walrus caps every instruction at 1 wait. Emit one drain per
        # proc instead, each carrying a single wait.
        from concourse.vector_clock import ScopedClock, VectorClock

        def _patched_drain_and_barrier(self, tick_clock, wait_clock):
            from concourse.tile_sem_assignment import tick_to_sem

            gc = tick_clock.global_clock
            n = len(gc)
            for p in range(n):
                if gc[p] > 0:
                    partial = VectorClock([gc[q] if q == p else 0 for q in range(n)])
                    d = self.nc.sync.drain()
                    wait_clock.add_sem_waits(d.ins, ScopedClock({None: partial}))
            # The stock tail is drain + all-engine barrier + RANGE_CLEAR +
            # barrier. The RANGE_CLEAR doesn't codegen on this walrus (NRT
            # re-zeroes sems per execution instead, verified in test.py), and
            # with no sem clear the two barrier butterflies guard nothing:
            # the per-proc drains above already hold the SP queue until every
            # proc -- including the output-DMA lanes -- reached its final
            # tick. Dropping both barriers saves ~4 us of fixed tail.
            assert self.sems is not None
            popped = self.nc._tile_sem_poison_stack.pop()
            assert popped is self._sem_poison
            self.nc.clear_and_free_semaphores(list(self.sems.allocated().values()))

        tc._drain_and_barrier = types.MethodType(_patched_drain_and_barrier, tc)

        singles = ctx.enter_context(tc.tile_pool(name="singles", bufs=1))
        dpool = ctx.enter_context(tc.tile_pool(name="d", bufs=nch))
        mpool = ctx.enter_context(tc.tile_pool(name="mask", bufs=nch))
        apool = ctx.enter_context(tc.tile_pool(name="a", bufs=nidx))
        spool = ctx.enter_context(tc.tile_pool(name="scr", bufs=nidx))
        psum_pool = ctx.enter_context(tc.tile_pool(name="psum", bufs=3, space="PSUM"))
        warm_pool = ctx.enter_context(tc.tile_pool(name="warm", bufs=1, space="PSUM"))

        consts = singles.tile([P, 4, P], BF16)
        nc.sync.dma_start(consts[:], consts_d.ap())
        T4ap = consts[:, 0, :]
        Iap = consts[:, 1, :]
        Etop = consts[:, 2, :]
        Ebot = consts[:, 3, :]

        # per-engine partials tiles (separate so DVE and ACT never WAW-collide)
        partials_s = singles.tile([P, nidx], F32)
        partials_m = singles.tile([P, nidx], F32)
        # DVE clock-carrier scratch: one distinct cell per carrier op
        dscr = singles.tile([1, nch], BF16)

        # absorb the consts-DMA wait on the PE engine early
        warm = warm_pool.tile([1, 2], F32)
        nc.tensor.matmul(warm[0:1, 0:1], Iap[:, 0:1], Iap[:, 0:1], start=True, stop=True)

        # ---- loads: d chunks via SWDGE cast+accumulate, mask chunks via HWDGE ----
        # chunk index: (g, i) -> g * b + i, covering image i rows [g*ct*P, (g+1)*ct*P)
        # Each targ-accum DMA waits for its pred DMA to complete, which blocks
        # the SWDGE queue behind it — so issue the pred DMAs two ahead of the
        # targ DMAs to keep two transfers in flight during the waits.
        chunk_order = [(g, i) for g in range(ngrp) for i in range(b)]
        d_chunks = {}
        mask_chunks = {}

        def _emit_pred(c):
            g, i = c
            rs = g * ct * P
            dt = dpool.tile([P, ct, w], BF16)
            src = pred_ap[i, rs:rs + ct * P, :].rearrange("(t p) w -> p t w", p=P)
            nc.gpsimd.dma_start(dt[:], src)                  # d := pred (cast bf16)
            d_chunks[c] = dt

        targ_insts = {}

        def _emit_targ(c):
            g, i = c
            rs = g * ct * P
            src2 = targ_ap[i, rs:rs + ct * P, :].rearrange("(t p) w -> p t w", p=P)
            # CCE only supports add in Copy mode: host passes -target, so
            # this computes d = pred + (-target)
            targ_insts[c] = nc.gpsimd.dma_start(
                d_chunks[c][:], src2, accum_op=mybir.AluOpType.add)

        lookahead = min(2, len(chunk_order))
        for c in chunk_order[:lookahead]:
            _emit_pred(c)
        for k, c in enumerate(chunk_order):
            _emit_targ(c)
            if k + lookahead < len(chunk_order):
                _emit_pred(chunk_order[k + lookahead])
        for g in range(ngrp):
            rs = g * ct * P
            for i in range(b):
                mt = mpool.tile([P, ct, w], F32)
                mi = nc.sync.dma_start(
                    mt[:], mask_ap[i, rs:rs + ct * P, :].rearrange("(t p) w -> p t w", p=P))
                # hold each mask chunk behind its d chunk's accumulate: the d
                # path is the critical one, so don't let mask traffic share
                # SDMA bandwidth with it early on (mask DMAs have a free wait
                # slot; TTR/mcopy consume mask much later anyway)
                tile.add_dep_helper(mi.ins, targ_insts[(g, i)].ins, sync=True,
                                    reason="mask after d chunk")
                mask_chunks[(g, i)] = mt
                # DVE clock carrier for this mask DMA (so TTRs need no mask wait)
                q = g * b + i
                nc.vector.tensor_copy(dscr[0:1, q:q + 1], mt[0:1, 0, 0:1])

        def dseg(t, i):
            return d_chunks[(t // ct, i)][:, t % ct, :]

        def mseg(t, i):
            return mask_chunks[(t // ct, i)][:, t % ct, :]

        # ---- per 128-row block: laplacian on PE, masked abs-sum on DVE,
        # ---- mask sum on ACT ----
        am_tiles = []
        for pos, (t, i) in enumerate((t, i) for t in range(nt) for i in range(b)):
            if True:
                idx = t * b + i
                seg = dseg(t, i)
                # PE clock carrier 1: observe the newest d chunk this group
                # needs (1 wait on its targ-DMA lane)
                tn = min(t + 1, nt - 1)
                nc.tensor.matmul(
                    warm[0:1, 0:1], Iap[:, 0:1], dseg(tn, i)[:, 0:1],
                    start=True, stop=True,
                )
                if pos >= 3:
                    # PE clock carrier 2: observe the DVE read of the psum
                    # slot this group will reuse (bufs=3), so the first real
                    # matmul only carries its PE-pipeline WAW wait
                    nc.tensor.matmul(
                        warm[0:1, 1:2], Iap[:, 0:1], am_tiles[pos - 3][:, 0:1],
                        start=True, stop=True,
                    )
                psum = psum_pool.tile([P, w], F32)
                for hb in range(0, w, 512):
                    e = hb + 512
                    # (out_range, lhsT, rhs_range) - all accumulate into psum[:, hb:e]
                    mms = [((hb, e), T4ap, seg[:, hb:e])]
                    ls = max(hb, 1)  # left neighbor: out col j <- d col j-1, j >= 1
                    mms.append(((ls, e), Iap, seg[:, ls - 1:e - 1]))
                    re = min(e, w - 1)  # right: out col j <- d col j+1, j <= w-2
                    mms.append(((hb, re), Iap, seg[:, hb + 1:re + 1]))
                    if t > 0:
                        mms.append(((hb, e), Etop, dseg(t - 1, i)[:, hb:e]))
                    if t < nt - 1:
                        mms.append(((hb, e), Ebot, dseg(t + 1, i)[:, hb:e]))
                    for j, ((o0, o1), lhsT, rhs) in enumerate(mms):
                        nc.tensor.matmul(
                            psum[:, o0:o1], lhsT, rhs,
                            start=(j == 0), stop=(j == len(mms) - 1),
                            skip_group_check=True,
                        )
                # masked abs-sum: am = psum * mask (TT), then
                # reduce-add with |.| -> partials_s column. (|lap*mask| ==
                # |lap|*mask for the nonnegative masks this loss uses.)
                am = apool.tile([P, w], BF16)
                nc.vector.tensor_tensor(am[:], psum[:], mseg(t, i), mybir.AluOpType.mult)
                am_tiles.append(am)
                nc.vector.tensor_reduce(
                    partials_s[:, idx:idx + 1], am[:], mybir.AxisListType.X,
                    op=mybir.AluOpType.add, apply_absolute_value=True,
                )
                scr2 = spool.tile([P, w], BF16, tag="scr2")
                nc.scalar.activation(
                    scr2[:], mseg(t, i),
                    mybir.ActivationFunctionType.Abs,
                    accum_out=partials_m[:, idx:idx + 1],
                )

        nc.sync.dma_start(out_d.ap()[:, 0:nidx], partials_s[:])
        nc.sync.dma_start(out_d.ap()[:, nidx:2 * nidx], partials_m[:])

    return nc


_NC_CACHE = {}


def _get_nc(b, h, w):
    key = (b, h, w)
    if key not in _NC_CACHE:
        _NC_CACHE[key] = build_nc(b, h, w)
    return _NC_CACHE[key]


def make_in_maps(pred, target, mask, n_cores=N_CORES):
    pred = np.ascontiguousarray(np.asarray(pred, dtype=np.float32)).reshape(B_TOTAL, H, W)
    # negated: the kernel folds d = pred - target into the target-load DMA via
    # CCE accumulate, which only supports add
    target = -np.asarray(target, dtype=np.float32).reshape(B_TOTAL, H, W)
    mask = np.ascontiguousarray(np.asarray(mask, dtype=np.float32)).reshape(B_TOTAL, H, W)
    import ml_dtypes
    consts = make_consts(ml_dtypes.bfloat16)
    bpc = B_TOTAL // n_cores
    in_maps = []
    for c in range(n_cores):
        in_maps.append({
            "pred": pred[c * bpc:(c + 1) * bpc],
            "target": target[c * bpc:(c + 1) * bpc],
            "mask": mask[c * bpc:(c + 1) * bpc],
            "consts": consts,
        })
    return in_maps


def combine(results):
    nidx = (H // P) * B
    S = 0.0
    M = 0.0
    for r in results:
        p = r["partials"].astype(np.float64)
        S += p[:, :nidx].sum()
        M += p[:, nidx:].sum()
    return np.float32(S / (M + 1e-8))


def kernel(pred, target, mask):
    nc = _get_nc(B, H, W)
    in_maps = make_in_maps(pred, target, mask)
    res = run_bass_kernel_spmd(nc, in_maps, core_ids=list(range(N_CORES)))
    out = combine(res.results)
    return np.array(out, dtype=np.float32)

